# revision 5
# baseline (speedup 1.0000x reference)
"""Chamfer loss (bidirectional, mean) on 8 trn2 NeuronCores.

pred/target: (16, 4096, 3) fp32.  Data-parallel over batch: 2 batches/core.

Math: for s = -d^2 = 2 p.q - |p|^2 - |q|^2, both chamfer directions are
max-reductions of s, computed per 128x4096 residency produced by K=18
augmented matmuls in split-bf16 (hi/lo) precision (see make_in_maps).
The emulated end-to-end error vs fp64 is ~1e-6 relative.

v2 pipeline (the 350us baseline serialized PE -> full-PSUM drain -> PE
on the single 8-bank residency, cycle = drain+PE = 5.4us/tile):
  - PSUM is split into two (128,2048) half-residency slots that
    ping-pong: ScalarE drains one half (a single 2048-wide copy,
    1.85us) while PE fills the other (4x 512-wide matmuls, 0.85us), so
    ScalarE streams back-to-back and PE never blocks the drain.
  - The two batches are interleaved tile-by-tile (A0,B0,A1,B1,...)
    so consecutive DVE ops belong to independent dependency chains;
    this hides the DVE pipe-drain bubble between dependent ops
    (measured ~500ns on serial in-place chains).
  - Row (pred-side) max per tile: bf16 2x tt-max tree
    4096->2048->1024->512->256 into a per-8-tile row8 buffer, plus one
    batched tensor_reduce per 8 tiles. (tensor_mask_reduce and
    tensor_tensor_reduce both crash this machine's DVE ucode; GpSimd
    TensorTensor is rejected by neuronxcc, so the tree stays.)
  - Col (target-side): running cm = max(cm, dr) bf16 tt per tile; at
    batch end PE transposes cm into PSUM, ScalarE copies it back to
    SBUF (tail slack), and a 2x tt-max tree reduces over the pred axis.
DVE is the bottleneck at ~315us of ops; ScalarE ~237us; PE ~150us.
"""

import sys

sys.path.insert(0, "/opt/trn_rl_repo")

import numpy as np
import ml_dtypes

import concourse.bass as bass
import concourse.tile as tile
from concourse import bacc, mybir
from concourse.bass_utils import run_bass_kernel_spmd

BF16 = ml_dtypes.bfloat16

N_CORES = 8
B = 16
N = 4096  # points per cloud
BPC = B // N_CORES  # batches per core
NT = N // 128  # 32 pred tiles per batch


def build_kernel(nc: bass.Bass, tc: "tile.TileContext", ctx):
    f32 = mybir.dt.float32
    bf16 = mybir.dt.bfloat16
    AF = mybir.ActivationFunctionType
    OP = mybir.AluOpType
    X = mybir.AxisListType.X

    augp_d = nc.dram_tensor("augp", [BPC, 18, N], bf16, kind="ExternalInput").ap()
    augt_d = nc.dram_tensor("augt", [BPC, 18, N], bf16, kind="ExternalInput").ap()
    eye_d = nc.dram_tensor("eye", [128, 128], bf16, kind="ExternalInput").ap()
    out_d = nc.dram_tensor("out", [1, 1], f32, kind="ExternalOutput").ap()

    const_p = ctx.enter_context(tc.tile_pool(name="const", bufs=1))
    aug_p = ctx.enter_context(tc.tile_pool(name="aug", bufs=2))
    nrm_p = ctx.enter_context(tc.tile_pool(name="nrm", bufs=2))
    dr_p = ctx.enter_context(tc.tile_pool(name="dr", bufs=5))
    s8_p = ctx.enter_context(tc.tile_pool(name="s8", bufs=2))
    tr_p = ctx.enter_context(tc.tile_pool(name="tr", bufs=2))
    cm_p = ctx.enter_context(tc.tile_pool(name="cm", bufs=2))
    rm_p = ctx.enter_context(tc.tile_pool(name="rm", bufs=2))
    fin_p = ctx.enter_context(tc.tile_pool(name="fin", bufs=2))
    ps_p = ctx.enter_context(tc.tile_pool(name="ps", bufs=2, space="PSUM"))

    eye = const_p.tile([128, 128], bf16, tag="eye")
    nc.sync.dma_start(eye[:], eye_d)
    ones = const_p.tile([128, 1], f32, tag="ones")
    nc.vector.memset(ones[:], 1.0)
    total = const_p.tile([128, 1], f32, tag="total")
    nc.vector.memset(total[:], 0.0)
    # warm ScalarE's activation table (Copy set) during input DMAs
    warmc = const_p.tile([128, 1], bf16, tag="warmc")
    nc.scalar.copy(warmc[:], ones[:])

    def prep_batch(b):
        """DMA aug seeds, compute norm rows 12-14 (pred) and 15-17 (target)."""
        augp = aug_p.tile([18, N], bf16, tag="augp")
        augt = aug_p.tile([18, N], bf16, tag="augt")
        nc.sync.dma_start(augp[:], augp_d[b])
        nc.sync.dma_start(augt[:], augt_d[b])

        for (aug, dram, scale, hr, lr, r0) in (
            (augp, augp_d, 0.5, 0, 6, 12),  # coords shipped as 2*hi / 2*lo
            (augt, augt_d, 1.0, 0, 3, 15),
        ):
            hi96 = nrm_p.tile([128, 96], bf16, tag="hi96")
            lo96 = nrm_p.tile([128, 96], bf16, tag="lo96")
            nc.sync.dma_start(
                hi96[:], dram[b, hr : hr + 3, :].rearrange("c (p u) -> p c u", p=128)
            )
            nc.sync.dma_start(
                lo96[:], dram[b, lr : lr + 3, :].rearrange("c (p u) -> p c u", p=128)
            )
            # all-DVE norm chain: avoids ACT hops + Square table load at startup
            c96 = nrm_p.tile([128, 96], f32, tag="c96")
            nc.vector.tensor_tensor(c96[:], hi96[:], lo96[:], OP.add)
            sq96 = nrm_p.tile([128, 96], f32, tag="sq96")
            nc.vector.tensor_tensor(sq96[:], c96[:], c96[:], OP.mult)
            nrm = nrm_p.tile([128, 32], f32, tag="nrm")
            nc.vector.tensor_reduce(
                nrm[:], sq96[:].rearrange("p (c u) -> p u c", c=3), axis=X, op=OP.add
            )
            nneg = nrm_p.tile([128, 32], f32, tag="nneg")
            nc.vector.tensor_scalar_mul(nneg[:], nrm[:], -scale * scale)
            nh = nrm_p.tile([128, 32], bf16, tag="nh")
            nc.vector.tensor_copy(nh[:], nneg[:])
            r1 = nrm_p.tile([128, 32], f32, tag="r1")
            nc.vector.tensor_tensor(r1[:], nneg[:], nh[:], OP.subtract)
            nm = nrm_p.tile([128, 32], bf16, tag="nm")
            nc.vector.tensor_copy(nm[:], r1[:])
            nl = nrm_p.tile([128, 32], bf16, tag="nl")
            nc.vector.tensor_tensor(nl[:], r1[:], nm[:], OP.subtract)
            # scatter (128,32) -> aug rows r0 (hi), r0+1 (mid), r0+2 (lo)
            for off, part in ((0, nh), (1, nm), (2, nl)):
                nc.sync.dma_start(
                    aug[r0 + off : r0 + off + 1, :].rearrange(
                        "o (p u) -> o p u", p=128
                    ),
                    part[:],
                )
        return augp, augt

    class BatchState:
        def __init__(self, b):
            self.b = b
            self.rm = rm_p.tile([128, 32], f32, tag="rm")
            self.cm = cm_p.tile([128, N], bf16, tag="cm")
            self.cm_init = False
            self.row8 = None

    def tile_step(st: BatchState, augp, augt, i):
        """One pred tile: matmuls (half-residency ping-pong), drain, tree, fold."""
        lhsT = augp[:, bass.ts(i, 128)]
        dr = dr_p.tile([128, N], bf16, tag="dr")
        for h in range(2):
            ps = ps_p.tile([128, 2048], f32, tag="ps")
            for k in range(4):
                nc.tensor.matmul(
                    ps[:, k * 512 : (k + 1) * 512],
                    lhsT,
                    augt[:, h * 2048 + k * 512 : h * 2048 + (k + 1) * 512],
                    start=True,
                    stop=True,
                )
            nc.scalar.copy(dr[:, h * 2048 : (h + 1) * 2048], ps[:])

        # pred-side row max: bf16 2x tt-max tree into row8, reduce per 8
        scr = tr_p.tile([128, 3584], bf16, tag="scr", bufs=3)
        nc.vector.tensor_tensor(scr[:, 0:2048], dr[:, 0:2048], dr[:, 2048:4096], OP.max)
        nc.vector.tensor_tensor(
            scr[:, 2048:3072], scr[:, 0:1024], scr[:, 1024:2048], OP.max
        )
        nc.vector.tensor_tensor(
            scr[:, 3072:3584], scr[:, 2048:2560], scr[:, 2560:3072], OP.max
        )
        g = i % 8
        if g == 0:
            st.row8 = s8_p.tile([128, 2048], bf16, tag=f"row8_{st.b}")
        nc.vector.tensor_tensor(
            st.row8[:, g * 256 : (g + 1) * 256],
            scr[:, 3072:3328],
            scr[:, 3328:3584],
            OP.max,
        )
        if g == 7:
            nc.vector.tensor_reduce(
                st.rm[:, i - 7 : i + 1],
                st.row8[:].rearrange("p (k u) -> p k u", k=8),
                axis=X,
                op=OP.max,
            )
        # target-side running fold
        if not st.cm_init:
            nc.vector.tensor_copy(st.cm[:], dr[:])
            st.cm_init = True
        else:
            nc.vector.tensor_tensor(st.cm[:], st.cm[:], dr[:], OP.max)

    def finalize_batch(st: BatchState):
        """Adds the batch's two direction-sums into `total`."""
        # pred side: sqrt(relu(-max)) summed per partition
        rr = rm_p.tile([128, 32], f32, tag="rr")
        nc.scalar.activation(rr[:], st.rm[:], AF.Relu, scale=-1.0)
        rs = rm_p.tile([128, 32], f32, tag="rs")
        nc.scalar.activation(rs[:], rr[:], AF.Sqrt)
        rsum = fin_p.tile([128, 1], f32, tag="rsum")
        nc.vector.tensor_reduce(rsum[:], rs[:], axis=X, op=OP.add)
        nc.vector.tensor_tensor(total[:], total[:], rsum[:], OP.add)

        # target side: transpose cm blocks on PE, ScalarE copies back to
        # SBUF, then a 2x bf16 tree reduces the (now free-dim) pred axis.
        psT = ps_p.tile([128, N], bf16, tag="ps")
        for k in range(NT):
            nc.tensor.transpose(
                psT[:, k * 128 : (k + 1) * 128],
                st.cm[:, k * 128 : (k + 1) * 128],
                eye[:],
            )
        cmT = tr_p.tile([128, 4096], bf16, tag="cmT")
        nc.scalar.copy(cmT[:, 0:2048], psT[:, 0:2048])
        nc.scalar.copy(cmT[:, 2048:4096], psT[:, 2048:4096])
        # tree over the 128-wide blocks: (32 blocks, 128) -> (32, 1)
        v = cmT[:].rearrange("p (t f) -> p t f", t=NT)
        w = 64
        while w >= 32:
            nc.vector.tensor_tensor(v[:, :, 0:w], v[:, :, 0:w], v[:, :, w : 2 * w], OP.max)
            w //= 2
        # remaining 32 -> 1 per block via strided reduce (32*32=1024 elems)
        cmax32 = rm_p.tile([128, 32], f32, tag="cmax32")
        nc.vector.tensor_reduce(cmax32[:], v[:, :, 0:32], axis=X, op=OP.max)
        cr = rm_p.tile([128, 32], f32, tag="cr")
        nc.scalar.activation(cr[:], cmax32[:], AF.Relu, scale=-1.0)
        cs = rm_p.tile([128, 32], f32, tag="cs")
        nc.scalar.activation(cs[:], cr[:], AF.Sqrt)
        csum = fin_p.tile([128, 1], f32, tag="csum")
        nc.vector.tensor_reduce(csum[:], cs[:], axis=X, op=OP.add)
        nc.vector.tensor_tensor(total[:], total[:], csum[:], OP.add)

    # PE warm-up: dummy matmuls on the eye tile while aug prep DMAs/norms
    # run, so the HAM clock-gate opens before the real loop.
    wps = ps_p.tile([128, 512], f32, tag="ps")
    for w in range(24):
        nc.tensor.matmul(wps[:, 0:128], eye[:], eye[:], start=True, stop=True)

    preps = [prep_batch(b) for b in range(BPC)]
    states = [BatchState(b) for b in range(BPC)]
    # interleave the two batches tile-by-tile to break DVE dependency chains
    for i in range(NT):
        for b in range(BPC):
            tile_step(states[b], *preps[b], i)
    for b in range(BPC):
        finalize_batch(states[b])

    # ---- final partition sum via matmul with ones, then DMA out
    psF = ps_p.tile([1, 1], f32, tag="ps")
    nc.tensor.matmul(psF[:], total[:], ones[:], start=True, stop=True)
    outsb = fin_p.tile([1, 1], f32, tag="outsb")
    nc.vector.tensor_copy(outsb[:], psF[:])
    nc.sync.dma_start(out_d, outsb[:])


_COMPILED = None


def _get_compiled():
    global _COMPILED
    if _COMPILED is None:
        from contextlib import ExitStack

        nc = bacc.Bacc(
            "TRN2", target_bir_lowering=False, debug=False, num_devices=N_CORES
        )
        with tile.TileContext(nc) as tc:
            with ExitStack() as ctx:
                build_kernel(nc, tc, ctx)
        nc.compile()
        _COMPILED = nc
    return _COMPILED


def _split_hi_lo(x):
    hi = x.astype(BF16)
    lo = (x - hi.astype(np.float32)).astype(BF16)
    return hi, lo


def make_in_maps(pred, target):
    pred = np.asarray(pred, dtype=np.float32)
    target = np.asarray(target, dtype=np.float32)
    eye = np.eye(128, dtype=BF16)
    in_maps = []
    for c in range(N_CORES):
        sl = slice(c * BPC, (c + 1) * BPC)
        p = np.ascontiguousarray(pred[sl].transpose(0, 2, 1))  # (BPC, 3, N)
        t = np.ascontiguousarray(target[sl].transpose(0, 2, 1))
        ph, pl = _split_hi_lo(p)
        th, tl = _split_hi_lo(t)
        augp = np.zeros((BPC, 18, N), dtype=BF16)
        augt = np.zeros((BPC, 18, N), dtype=BF16)
        augp[:, 0:3] = (ph.astype(np.float32) * 2.0).astype(BF16)
        augp[:, 3:6] = augp[:, 0:3]
        augp[:, 6:9] = (pl.astype(np.float32) * 2.0).astype(BF16)
        augp[:, 9:12] = augp[:, 6:9]
        augp[:, 15:18] = np.ones((BPC, 3, N), dtype=BF16)
        augt[:, 0:3] = th
        augt[:, 3:6] = tl
        augt[:, 6:9] = th
        augt[:, 9:12] = tl
        augt[:, 12:15] = np.ones((BPC, 3, N), dtype=BF16)
        in_maps.append({"augp": augp, "augt": augt, "eye": eye})
    return in_maps


def _ensure_ntff_hook():
    """This container's antenv lacks axon_hooks; synthesize it from the
    boot helper so run_bass_kernel_spmd(trace=True) can capture NTFFs."""
    try:
        import antenv.axon_hooks  # noqa: F401

        return
    except ImportError:
        pass
    import types

    import antenv
    from trn_agent_boot.trn_boot import _ntff_profile_via_ctypes

    hook = _ntff_profile_via_ctypes("/opt/axon/libaxon_pjrt.so")
    mod = types.ModuleType("antenv.axon_hooks")
    mod.get_axon_ntff_profile_hook = lambda: hook
    mod.set_axon_ntff_profile_hook = lambda h: None
    sys.modules["antenv.axon_hooks"] = mod
    antenv.axon_hooks = mod


def run(pred, target, trace=False):
    if trace:
        try:
            _ensure_ntff_hook()
        except Exception as e:
            print(f"ntff hook setup failed ({e}); running untraced")
            trace = False
    nc = _get_compiled()
    in_maps = make_in_maps(pred, target)
    res = run_bass_kernel_spmd(
        nc, in_maps, core_ids=list(range(N_CORES)), trace=trace
    )
    parts = [float(res.results[c]["out"][0, 0]) for c in range(N_CORES)]
    val = np.float32(sum(parts) / (B * N * 2.0))
    return val, res


def kernel(pred, target):
    val, _ = run(pred, target)
    return np.array(val, dtype=np.float32)


# revision 11
# speedup vs baseline: 1.0523x; 1.0523x over previous
"""Chamfer loss (bidirectional, mean) on 8 trn2 NeuronCores.

pred/target: (16, 4096, 3) fp32.  Data-parallel over batch: 2 batches/core.

Math: for s = -d^2 = 2 p.q - |p|^2 - |q|^2, both chamfer directions are
max-reductions of s, computed per 128x4096 residency produced by K=18
augmented matmuls in split-bf16 (hi/lo) precision (see make_in_maps).
The emulated end-to-end error vs fp64 is ~1e-6 relative.

v2 pipeline (the 350us baseline serialized PE -> full-PSUM drain -> PE
on the single 8-bank residency, cycle = drain+PE = 5.4us/tile):
  - PSUM is split into two (128,2048) half-residency slots that
    ping-pong: ScalarE drains one half (a single 2048-wide copy,
    1.85us) while PE fills the other (4x 512-wide matmuls, 0.85us), so
    ScalarE streams back-to-back and PE never blocks the drain.
  - The two batches are interleaved tile-by-tile (A0,B0,A1,B1,...)
    so consecutive DVE ops belong to independent dependency chains;
    this hides the DVE pipe-drain bubble between dependent ops
    (measured ~500ns on serial in-place chains).
  - Row (pred-side) max per tile: bf16 2x tt-max tree
    4096->2048->1024->512->256 into a per-8-tile row8 buffer, plus one
    batched tensor_reduce per 8 tiles. (tensor_mask_reduce and
    tensor_tensor_reduce both crash this machine's DVE ucode; GpSimd
    TensorTensor is rejected by neuronxcc, so the tree stays.)
  - Col (target-side): running cm = max(cm, dr) bf16 tt per tile; at
    batch end PE transposes cm into PSUM, ScalarE copies it back to
    SBUF (tail slack), and a 2x tt-max tree reduces over the pred axis.
DVE is the bottleneck at ~315us of ops; ScalarE ~237us; PE ~150us.
"""

import sys

sys.path.insert(0, "/opt/trn_rl_repo")

import numpy as np
import ml_dtypes

import concourse.bass as bass
import concourse.tile as tile
from concourse import bacc, mybir
from concourse.bass_utils import run_bass_kernel_spmd

BF16 = ml_dtypes.bfloat16

N_CORES = 8
B = 16
N = 4096  # points per cloud
BPC = B // N_CORES  # batches per core
NT = N // 128  # 32 pred tiles per batch


def build_kernel(nc: bass.Bass, tc: "tile.TileContext", ctx):
    f32 = mybir.dt.float32
    bf16 = mybir.dt.bfloat16
    AF = mybir.ActivationFunctionType
    OP = mybir.AluOpType
    X = mybir.AxisListType.X

    augp_d = nc.dram_tensor("augp", [BPC, 18, N], bf16, kind="ExternalInput").ap()
    augt_d = nc.dram_tensor("augt", [BPC, 18, N], bf16, kind="ExternalInput").ap()
    eye_d = nc.dram_tensor("eye", [128, 128], bf16, kind="ExternalInput").ap()
    out_d = nc.dram_tensor("out", [1, 1], f32, kind="ExternalOutput").ap()

    const_p = ctx.enter_context(tc.tile_pool(name="const", bufs=1))
    aug_p = ctx.enter_context(tc.tile_pool(name="aug", bufs=2))
    dr_p = ctx.enter_context(tc.tile_pool(name="dr", bufs=5))
    s8_p = ctx.enter_context(tc.tile_pool(name="s8", bufs=2))
    tr_p = ctx.enter_context(tc.tile_pool(name="tr", bufs=2))
    cm_p = ctx.enter_context(tc.tile_pool(name="cm", bufs=2))
    rm_p = ctx.enter_context(tc.tile_pool(name="rm", bufs=2))
    fin_p = ctx.enter_context(tc.tile_pool(name="fin", bufs=2))
    ps_p = ctx.enter_context(tc.tile_pool(name="ps", bufs=2, space="PSUM"))

    eye = const_p.tile([128, 128], bf16, tag="eye")
    nc.sync.dma_start(eye[:], eye_d)
    ones = const_p.tile([128, 1], f32, tag="ones")
    nc.vector.memset(ones[:], 1.0)
    total = const_p.tile([128, 1], f32, tag="total")
    nc.vector.memset(total[:], 0.0)
    # warm ScalarE's activation tables during input DMAs: Sqrt set first
    # (covers Sqrt+Relu+Copy for the whole kernel -> no later table load)
    warmc = const_p.tile([128, 1], f32, tag="warmc")
    nc.scalar.activation(warmc[:], ones[:], AF.Sqrt)
    nc.scalar.copy(warmc[:], ones[:])

    def prep_batch(b):
        """DMA the aug tiles (norm rows are precomputed host-side)."""
        augp = aug_p.tile([18, N], bf16, tag="augp")
        augt = aug_p.tile([18, N], bf16, tag="augt")
        nc.sync.dma_start(augp[:], augp_d[b])
        nc.sync.dma_start(augt[:], augt_d[b])
        return augp, augt

    class BatchState:
        def __init__(self, b):
            self.b = b
            self.rm = rm_p.tile([128, 32], f32, tag="rm")
            self.cm = cm_p.tile([128, N], bf16, tag="cm")
            self.cm_init = False
            self.row8 = None

    def tile_step(st: BatchState, augp, augt, i):
        """One pred tile: matmuls (half-residency ping-pong), drain, tree, fold."""
        lhsT = augp[:, bass.ts(i, 128)]
        dr = dr_p.tile([128, N], bf16, tag="dr")
        for h in range(2):
            ps = ps_p.tile([128, 2048], f32, tag="ps")
            for k in range(4):
                nc.tensor.matmul(
                    ps[:, k * 512 : (k + 1) * 512],
                    lhsT,
                    augt[:, h * 2048 + k * 512 : h * 2048 + (k + 1) * 512],
                    start=True,
                    stop=True,
                )
            nc.scalar.copy(dr[:, h * 2048 : (h + 1) * 2048], ps[:])

        # pred-side row max: bf16 2x tt-max tree into row8, reduce per 8
        scr = tr_p.tile([128, 3584], bf16, tag="scr", bufs=3)
        nc.vector.tensor_tensor(scr[:, 0:2048], dr[:, 0:2048], dr[:, 2048:4096], OP.max)
        nc.vector.tensor_tensor(
            scr[:, 2048:3072], scr[:, 0:1024], scr[:, 1024:2048], OP.max
        )
        nc.vector.tensor_tensor(
            scr[:, 3072:3584], scr[:, 2048:2560], scr[:, 2560:3072], OP.max
        )
        g = i % 8
        if g == 0:
            st.row8 = s8_p.tile([128, 2048], bf16, tag=f"row8_{st.b}")
        nc.vector.tensor_tensor(
            st.row8[:, g * 256 : (g + 1) * 256],
            scr[:, 3072:3328],
            scr[:, 3328:3584],
            OP.max,
        )
        if g == 7:
            nc.vector.tensor_reduce(
                st.rm[:, i - 7 : i + 1],
                st.row8[:].rearrange("p (k u) -> p k u", k=8),
                axis=X,
                op=OP.max,
            )
        # target-side running fold
        if not st.cm_init:
            nc.vector.tensor_copy(st.cm[:], dr[:])
            st.cm_init = True
        else:
            nc.vector.tensor_tensor(st.cm[:], st.cm[:], dr[:], OP.max)

    def finalize_rm(st: BatchState):
        """pred side: sqrt(relu(-max)) on ScalarE (cheap, early)."""
        st.rr = rm_p.tile([128, 32], f32, tag="rr")
        nc.scalar.activation(st.rr[:], st.rm[:], AF.Relu, scale=-1.0)
        st.rs = rm_p.tile([128, 32], f32, tag="rs")
        nc.scalar.activation(st.rs[:], st.rr[:], AF.Sqrt)

    def finalize_cmtrans(st: BatchState):
        """target side: PE transposes cm into PSUM, ScalarE copies back."""
        psT = ps_p.tile([128, N], bf16, tag="ps")
        for k in range(NT):
            nc.tensor.transpose(
                psT[:, k * 128 : (k + 1) * 128],
                st.cm[:, k * 128 : (k + 1) * 128],
                eye[:],
            )
        st.cmT = tr_p.tile([128, 4096], bf16, tag="cmT")
        nc.scalar.copy(st.cmT[:, 0:2048], psT[:, 0:2048])
        nc.scalar.copy(st.cmT[:, 2048:4096], psT[:, 2048:4096])

    def finalize_dve(st: BatchState):
        """DVE reduces + adds both direction-sums into `total`."""
        rsum = fin_p.tile([128, 1], f32, tag="rsum")
        nc.vector.tensor_reduce(rsum[:], st.rs[:], axis=X, op=OP.add)
        nc.vector.tensor_tensor(total[:], total[:], rsum[:], OP.add)
        # tree over the 128-wide blocks: (32 blocks, 128) -> (32, 1)
        v = st.cmT[:].rearrange("p (t f) -> p t f", t=NT)
        w = 64
        while w >= 32:
            nc.vector.tensor_tensor(v[:, :, 0:w], v[:, :, 0:w], v[:, :, w : 2 * w], OP.max)
            w //= 2
        # remaining 32 -> 1 per block via strided reduce (32*32=1024 elems)
        cmax32 = rm_p.tile([128, 32], f32, tag="cmax32")
        nc.vector.tensor_reduce(cmax32[:], v[:, :, 0:32], axis=X, op=OP.max)
        cr = rm_p.tile([128, 32], f32, tag="cr")
        nc.scalar.activation(cr[:], cmax32[:], AF.Relu, scale=-1.0)
        cs = rm_p.tile([128, 32], f32, tag="cs")
        nc.scalar.activation(cs[:], cr[:], AF.Sqrt)
        csum = fin_p.tile([128, 1], f32, tag="csum")
        nc.vector.tensor_reduce(csum[:], cs[:], axis=X, op=OP.add)
        nc.vector.tensor_tensor(total[:], total[:], csum[:], OP.add)

    # PE warm-up: dummy matmuls on the eye tile while aug prep DMAs/norms
    # run, so the HAM clock-gate opens before the real loop.
    wps = ps_p.tile([128, 512], f32, tag="ps")
    for w in range(24):
        nc.tensor.matmul(wps[:, 0:128], eye[:], eye[:], start=True, stop=True)

    preps = [prep_batch(b) for b in range(BPC)]
    states = [BatchState(b) for b in range(BPC)]
    A, Bst = states
    # interleave the two batches tile-by-tile to break DVE dependency chains
    for i in range(NT - 1):
        for b in range(BPC):
            tile_step(states[b], *preps[b], i)
    # staggered tail: A's PE/ScalarE finalization overlaps B's last tile
    tile_step(A, *preps[0], NT - 1)
    finalize_rm(A)
    tile_step(Bst, *preps[1], NT - 1)
    finalize_cmtrans(A)
    finalize_rm(Bst)
    finalize_dve(A)
    finalize_cmtrans(Bst)
    finalize_dve(Bst)

    # ---- final partition sum via matmul with ones, then DMA out
    psF = ps_p.tile([1, 1], f32, tag="ps")
    nc.tensor.matmul(psF[:], total[:], ones[:], start=True, stop=True)
    outsb = fin_p.tile([1, 1], f32, tag="outsb")
    nc.vector.tensor_copy(outsb[:], psF[:])
    nc.sync.dma_start(out_d, outsb[:])


_COMPILED = None


def _get_compiled():
    global _COMPILED
    if _COMPILED is None:
        from contextlib import ExitStack

        nc = bacc.Bacc(
            "TRN2", target_bir_lowering=False, debug=False, num_devices=N_CORES
        )
        with tile.TileContext(nc) as tc:
            with ExitStack() as ctx:
                build_kernel(nc, tc, ctx)
        nc.compile()
        _COMPILED = nc
    return _COMPILED


def _split_hi_lo(x):
    hi = x.astype(BF16)
    lo = (x - hi.astype(np.float32)).astype(BF16)
    return hi, lo


def _split3(x):
    """Split fp64 (BPC, N) into three bf16 rows h/m/l with h+m+l ~= x."""
    h = x.astype(BF16)
    m = (x - h.astype(np.float64)).astype(BF16)
    l = (x - h.astype(np.float64) - m.astype(np.float64)).astype(BF16)
    return np.stack([h, m, l], axis=1)  # (BPC, 3, N)


def make_in_maps(pred, target):
    pred = np.asarray(pred, dtype=np.float32)
    target = np.asarray(target, dtype=np.float32)
    eye = np.eye(128, dtype=BF16)
    in_maps = []
    for c in range(N_CORES):
        sl = slice(c * BPC, (c + 1) * BPC)
        p = np.ascontiguousarray(pred[sl].transpose(0, 2, 1))  # (BPC, 3, N)
        t = np.ascontiguousarray(target[sl].transpose(0, 2, 1))
        ph, pl = _split_hi_lo(p)
        th, tl = _split_hi_lo(t)
        augp = np.zeros((BPC, 18, N), dtype=BF16)
        augt = np.zeros((BPC, 18, N), dtype=BF16)
        augp[:, 0:3] = (ph.astype(np.float32) * 2.0).astype(BF16)
        augp[:, 3:6] = augp[:, 0:3]
        augp[:, 6:9] = (pl.astype(np.float32) * 2.0).astype(BF16)
        augp[:, 9:12] = augp[:, 6:9]
        p_rec = ph.astype(np.float64) + pl.astype(np.float64)
        t_rec = th.astype(np.float64) + tl.astype(np.float64)
        augp[:, 12:15] = _split3(-np.square(p_rec).sum(axis=1))
        augp[:, 15:18] = np.ones((BPC, 3, N), dtype=BF16)
        augt[:, 0:3] = th
        augt[:, 3:6] = tl
        augt[:, 6:9] = th
        augt[:, 9:12] = tl
        augt[:, 12:15] = np.ones((BPC, 3, N), dtype=BF16)
        augt[:, 15:18] = _split3(-np.square(t_rec).sum(axis=1))
        in_maps.append({"augp": augp, "augt": augt, "eye": eye})
    return in_maps


def _ensure_ntff_hook():
    """This container's antenv lacks axon_hooks; synthesize it from the
    boot helper so run_bass_kernel_spmd(trace=True) can capture NTFFs."""
    try:
        import antenv.axon_hooks  # noqa: F401

        return
    except ImportError:
        pass
    import types

    import antenv
    from trn_agent_boot.trn_boot import _ntff_profile_via_ctypes

    hook = _ntff_profile_via_ctypes("/opt/axon/libaxon_pjrt.so")
    mod = types.ModuleType("antenv.axon_hooks")
    mod.get_axon_ntff_profile_hook = lambda: hook
    mod.set_axon_ntff_profile_hook = lambda h: None
    sys.modules["antenv.axon_hooks"] = mod
    antenv.axon_hooks = mod


def run(pred, target, trace=False):
    if trace:
        try:
            _ensure_ntff_hook()
        except Exception as e:
            print(f"ntff hook setup failed ({e}); running untraced")
            trace = False
    nc = _get_compiled()
    in_maps = make_in_maps(pred, target)
    res = run_bass_kernel_spmd(
        nc, in_maps, core_ids=list(range(N_CORES)), trace=trace
    )
    parts = [float(res.results[c]["out"][0, 0]) for c in range(N_CORES)]
    val = np.float32(sum(parts) / (B * N * 2.0))
    return val, res


def kernel(pred, target):
    val, _ = run(pred, target)
    return np.array(val, dtype=np.float32)
